# revision 1
# baseline (speedup 1.0000x reference)
"""Trainium2 Bass kernel for nn_CMP_3367254360436 (gnn_message_passing).

Reference computation: bidirectional signed scatter-add pooling over 8192
edges on 2048 nodes of [16,32,32] fp32 feature maps, concat [feats, pooled_pos,
pooled_neg] (48 ch), then three 3x3 SAME convs (48->32->32->16) with leaky
ReLU (0.1).

Device decomposition (per NeuronCore, 256 nodes/core in 64 quads of 4 nodes):
  1. Pooling: one dma_gather per quad pulls (contribution, channel) rows
     (idx = src_node*16 + ch, 4KB elements) from the full feats array into
     [128 rows, G, 1024] SBUF; compile-time 0/1 selection matrices S then
     accumulate rows into pooled (node, sign, ch) slots via fp32r matmuls in
     PSUM. ACT evacuates into a guarded tile P [128 = 4n x 32ch, 33+1024+33].
  2. Convs: 3x3 conv = 9 shifted flat-window matmuls with block-diagonal
     (4-node) weights accumulating in PSUM. Taps are split into 3 groups by
     kx so all matmuls of a PSUM accumulation group write identical bytes;
     the wrapped boundary column of groups kx=0 / kx=2 is zeroed afterwards
     with a strided memset. Groups are combined on DVE; bias + leaky ReLU on
     ACT (Lrelu) writes the next guarded tile.
  3. conv3 output [64 = 4n x 16ch, 1024] is DMA'd back to HBM.

The Bass program is identical on all 8 cores (SPMD); all per-core variation
(node assignment, S matrices, gather indices) is carried in the input data.
"""

import numpy as np

LAST_EXEC_TIME_NS = None
V, C, H, W = 2048, 16, 32, 32
NCORES = 8
NPQ = 4                      # nodes per quad
QPC = V // NCORES // NPQ     # quads per core = 64
GUARD = 33
GW = GUARD + 1024 + GUARD    # guarded tile free width = 1090


# ---------------------------------------------------------------- host prep

def _host_prep(feats, edges, W1, b1, W2, b2, W3, b3):
    edges = np.asarray(edges).reshape(-1, 3)
    src, sign, dst = edges[:, 0], edges[:, 1], edges[:, 2]
    feats = np.ascontiguousarray(np.asarray(feats), dtype=np.float32)

    pos = [[] for _ in range(V)]
    neg = [[] for _ in range(V)]
    for s, sg, d in zip(src, sign, dst):
        buck = pos if sg > 0 else neg
        buck[int(d)].append(int(s))
        buck[int(s)].append(int(d))

    wgt = np.array([len(pos[v]) + len(neg[v]) for v in range(V)])

    # degree-balanced quads: snake-deal sorted nodes into NCORES*QPC quads,
    # then deal quads (sorted by weight) across cores per slot so per-slot
    # group counts line up across cores.
    order = np.argsort(-wgt, kind="stable")
    nquads = NCORES * QPC
    quads = [[] for _ in range(nquads)]
    for i, v in enumerate(order):
        r, j = divmod(i, nquads)
        q = j if r % 2 == 0 else nquads - 1 - j
        quads[q].append(int(v))
    qw = [sum(wgt[v] for v in q) for q in quads]
    qorder = np.argsort(-np.array(qw), kind="stable")
    assign = np.array(qorder).reshape(QPC, NCORES)  # [slot, core] -> quad id

    slot_rows = {}
    for s in range(QPC):
        for c in range(NCORES):
            rows = []
            for n_local, v in enumerate(quads[assign[s, c]]):
                for sgn, lst in ((0, pos[v]), (1, neg[v])):
                    for u in lst:
                        for ch in range(16):
                            rows.append((u * 16 + ch, 32 * n_local + 16 * sgn + ch))
            slot_rows[(c, s)] = rows
    G = np.zeros(QPC, dtype=np.int64)
    for s in range(QPC):
        G[s] = max(1, max((len(slot_rows[(c, s)]) + 127) // 128
                          for c in range(NCORES)))
    Gtot = int(G.sum())
    goff = np.concatenate([[0], np.cumsum(G)]).astype(np.int64)

    in_maps = []
    node_lists = []
    for c in range(NCORES):
        idxs_pack = np.zeros((16, 8 * Gtot), np.int16)   # idx j -> [j%16, j//16]
        S_pack = np.zeros((Gtot * 128, 128), np.float32)
        f_own = np.zeros((QPC * 64, 1024), np.float32)
        nodes_c = []
        for s in range(QPC):
            rows = slot_rows[(c, s)]
            base = int(goff[s])
            for j, (srcidx, slot) in enumerate(rows):
                jj = base * 128 + j
                idxs_pack[jj % 16, jj // 16] = srcidx
                S_pack[base * 128 + j, slot] = 1.0
            nodes = quads[assign[s, c]]
            nodes_c.append(nodes)
            for n_local, v in enumerate(nodes):
                f_own[s * 64 + 16 * n_local: s * 64 + 16 * n_local + 16] = \
                    feats[v].reshape(16, 1024)
        node_lists.append(nodes_c)
        in_maps.append({
            "feats_all": feats.reshape(V * 16, 1024),
            "feats_own": f_own,
            # replicated across the 8 Q7 cores (16 partitions each)
            "idxs_pack": np.tile(idxs_pack, (8, 1)),
            "s_pack": S_pack,
        })

    # block-diag weight packs, stored as [K, 9*M] with tap t = 3*ky + kx
    W1 = np.asarray(W1); W2 = np.asarray(W2); W3 = np.asarray(W3)
    wa1 = np.zeros((128, 9, 128), np.float32)
    wb1 = np.zeros((64, 9, 128), np.float32)
    w2p = np.zeros((128, 9, 128), np.float32)
    w3p = np.zeros((128, 9, 64), np.float32)
    for ky in range(3):
        for kx in range(3):
            t = 3 * ky + kx
            for n in range(4):
                wa1[32*n:32*n+32, t, 32*n:32*n+32] = W1[:, 16:48, ky, kx].T
                wb1[16*n:16*n+16, t, 32*n:32*n+32] = W1[:, 0:16, ky, kx].T
                w2p[32*n:32*n+32, t, 32*n:32*n+32] = W2[:, :, ky, kx].T
                w3p[32*n:32*n+32, t, 16*n:16*n+16] = W3[:, :, ky, kx].T
    consts = {
        "wa1": wa1.reshape(128, 9 * 128), "wb1": wb1.reshape(64, 9 * 128),
        "w2p": w2p.reshape(128, 9 * 128), "w3p": w3p.reshape(128, 9 * 64),
        "b1t": np.tile(np.asarray(b1), 4).astype(np.float32).reshape(128, 1),
        "b2t": np.tile(np.asarray(b2), 4).astype(np.float32).reshape(128, 1),
        "b3t": np.tile(np.asarray(b3), 4).astype(np.float32).reshape(64, 1),
    }
    for m in in_maps:
        m.update({k: v.copy() for k, v in consts.items()})
    return in_maps, node_lists, G, goff, Gtot


# ------------------------------------------------------------- bass program

def _build_program(G, goff, Gtot, leaky_on_act=True, nslots=QPC,
                   for_sim=False, parts="all"):
    import concourse.mybir as mybir
    from concourse import bacc
    from concourse.tile import TileContext

    f32 = mybir.dt.float32
    f32r = mybir.dt.float32r
    nc = bacc.Bacc("TRN2", target_bir_lowering=False)

    feats_all = nc.dram_tensor("feats_all", [V * 16, 1024], f32r,
                               kind="ExternalInput")
    feats_own = nc.dram_tensor("feats_own", [QPC * 64, 1024], f32r,
                               kind="ExternalInput")
    idxs_pack = nc.dram_tensor("idxs_pack", [128, 8 * Gtot], mybir.dt.int16,
                               kind="ExternalInput")
    s_pack = nc.dram_tensor("s_pack", [Gtot * 128, 128], f32r,
                            kind="ExternalInput")
    wa1 = nc.dram_tensor("wa1", [128, 9 * 128], f32r, kind="ExternalInput")
    wb1 = nc.dram_tensor("wb1", [64, 9 * 128], f32r, kind="ExternalInput")
    w2p = nc.dram_tensor("w2p", [128, 9 * 128], f32r, kind="ExternalInput")
    w3p = nc.dram_tensor("w3p", [128, 9 * 64], f32r, kind="ExternalInput")
    b1t = nc.dram_tensor("b1t", [128, 1], f32, kind="ExternalInput")
    b2t = nc.dram_tensor("b2t", [128, 1], f32, kind="ExternalInput")
    b3t = nc.dram_tensor("b3t", [64, 1], f32, kind="ExternalInput")
    out_own = nc.dram_tensor("out_own", [QPC * 64, 1024], f32,
                             kind="ExternalOutput")

    # HW probe: Lrelu ignores the alpha operand (table slope 0.01);
    # Prelu honors alpha and matches leaky(0.1) exactly.
    LRELU = mybir.ActivationFunctionType.Prelu
    Gmax = int(G.max())


    with TileContext(nc) as tc:
        with (
            tc.tile_pool(name="const", bufs=1) as constp,
            tc.tile_pool(name="gath", bufs=3) as gathp,
            tc.tile_pool(name="stile", bufs=3) as stilep,
            tc.tile_pool(name="xt", bufs=3) as xtp,
            tc.tile_pool(name="comb", bufs=6) as combp,
            tc.tile_pool(name="otile", bufs=3) as otp,
            tc.tile_pool(name="poolps", bufs=1, space="PSUM") as poolpsp,
            tc.tile_pool(name="convps", bufs=6, space="PSUM") as convpsp,
        ):
            # ---- resident constants
            wa1_t = constp.tile([128, 9 * 128], f32r)
            wb1_t = constp.tile([64, 9 * 128], f32r)
            w2p_t = constp.tile([128, 9 * 128], f32r)
            w3p_t = constp.tile([128, 9 * 64], f32r)
            nc.sync.dma_start(out=wa1_t[:, :], in_=wa1[:, :])
            nc.sync.dma_start(out=wb1_t[:, :], in_=wb1[:, :])
            nc.sync.dma_start(out=w2p_t[:, :], in_=w2p[:, :])
            nc.sync.dma_start(out=w3p_t[:, :], in_=w3p[:, :])
            b1_t = constp.tile([128, 1], f32)
            b2_t = constp.tile([128, 1], f32)
            b3_t = constp.tile([64, 1], f32)
            nc.sync.dma_start(out=b1_t[:, :], in_=b1t[:, :])
            nc.sync.dma_start(out=b2_t[:, :], in_=b2t[:, :])
            nc.sync.dma_start(out=b3_t[:, :], in_=b3t[:, :])
            idx_t = constp.tile([128, 8 * Gtot], mybir.dt.int16)
            nc.sync.dma_start(out=idx_t[:, :], in_=idxs_pack[:, :])

            def conv_layer(x_tiles, w_tiles, Ks, M, bias, out_tile,
                           out_guarded):
                for w0 in (0, 512):
                    gps = []
                    for kx in range(3):
                        ps = convpsp.tile([128, 512], f32, tag="convps",
                                          name=f"ps_{kx}")
                        nmm = 3 * len(x_tiles)
                        i = 0
                        for ky in range(3):
                            t = 3 * ky + kx
                            delta = (ky - 1) * 32 + (kx - 1)
                            for xt, wt, K in zip(x_tiles, w_tiles, Ks):
                                a = GUARD + w0 + delta
                                nc.tensor.matmul(
                                    ps[:M, :],
                                    wt[:K, t * M:(t + 1) * M],
                                    xt[:K, a:a + 512],
                                    start=(i == 0), stop=(i == nmm - 1),
                                )
                                i += 1
                        # zero the wrapped boundary column
                        col = {0: 0, 2: 31}.get(kx)
                        if col is not None:
                            colap = ps[:M, :].rearrange(
                                "p (r c) -> p r c", c=32)[:, :, col:col + 1]
                            nc.vector.memset(colap, 0.0)
                        gps.append(ps)
                    s_t = combp.tile([128, 512], f32, tag="comb", name="s_t")
                    nc.vector.tensor_copy(out=s_t[:M, :], in_=gps[0][:M, :])
                    nc.vector.tensor_tensor(out=s_t[:M, :], in0=s_t[:M, :],
                                            in1=gps[1][:M, :],
                                            op=mybir.AluOpType.add)
                    nc.vector.tensor_tensor(out=s_t[:M, :], in0=s_t[:M, :],
                                            in1=gps[2][:M, :],
                                            op=mybir.AluOpType.add)
                    off = GUARD + w0 if out_guarded else w0
                    if leaky_on_act:
                        nc.scalar.activation(out_tile[:M, off:off + 512],
                                             s_t[:M, :], LRELU,
                                             bias=bias[:M, :], alpha=0.1)
                    else:
                        # leaky(x+b) = max(x+b, 0.1*(x+b)) on DVE
                        sb = combp.tile([128, 512], f32, tag="comb2",
                                        name="sb")
                        nc.vector.tensor_scalar(
                            out=sb[:M, :], in0=s_t[:M, :],
                            scalar1=bias[:M, :], scalar2=0.1,
                            op0=mybir.AluOpType.add,
                            op1=mybir.AluOpType.mult)
                        nc.vector.tensor_scalar(
                            out=s_t[:M, :], in0=s_t[:M, :],
                            scalar1=bias[:M, :], scalar2=None,
                            op0=mybir.AluOpType.add)
                        nc.vector.tensor_tensor(
                            out=out_tile[:M, off:off + 512], in0=s_t[:M, :],
                            in1=sb[:M, :], op=mybir.AluOpType.max)

            for s in range(nslots):
                g = int(G[s])
                base = int(goff[s])
                do_pool = parts in ("all", "pool")
                do_conv = parts in ("all", "conv")
                # ---- pooling gather: [128, g, 1024]
                if do_pool:
                  gath = gathp.tile([128, Gmax * 1024], f32r, tag="gath",
                                    name="gath")
                  nc.gpsimd.dma_gather(
                      out_ap=gath[:, :g * 1024].rearrange(
                          "p (gg f) -> p gg f", f=1024),
                      in_ap=feats_all[:, :],
                      idxs_ap=idx_t[:, base * 8:(base + g) * 8],
                      num_idxs=g * 128,
                      num_idxs_reg=g * 128,
                      elem_size=1024,
                      single_packet=False,
                  )
                  s_t = stilep.tile([128, Gmax * 128], f32r, tag="stile",
                                    name="s_mat")
                  nc.sync.dma_start(
                      out=s_t[:, :g * 128].rearrange("p (gg m) -> p gg m",
                                                     m=128),
                      in_=s_pack[base * 128:(base + g) * 128, :].rearrange(
                          "(gg p) m -> p gg m", p=128),
                  )
                  pool_ps = poolpsp.tile([128, 1024], f32, tag="poolps",
                                         name="pool_ps")
                  for w0 in (0, 512):
                      for gg in range(g):
                          nc.tensor.matmul(
                              pool_ps[:, w0:w0 + 512],
                              s_t[:, gg * 128:(gg + 1) * 128],
                              gath[:, gg * 1024 + w0:gg * 1024 + w0 + 512],
                              start=(gg == 0), stop=(gg == g - 1),
                          )
                # ---- guarded input tiles
                P = xtp.tile([128, GW], f32r, tag="P", name="P")
                nc.vector.memset(P[:, 0:GUARD].bitcast(f32), 0.0)
                nc.vector.memset(P[:, GUARD + 1024:GW].bitcast(f32), 0.0)
                if do_pool:
                    nc.vector.tensor_copy(out=P[:, GUARD:GUARD + 1024],
                                           in_=pool_ps[:, :])
                else:
                    nc.vector.memset(P[:, GUARD:GUARD + 1024].bitcast(f32),
                                     0.0)
                F = xtp.tile([64, GW], f32r, tag="F", name="F")
                nc.vector.memset(F[:, 0:GUARD].bitcast(f32), 0.0)
                nc.vector.memset(F[:, GUARD + 1024:GW].bitcast(f32), 0.0)
                nc.sync.dma_start(out=F[:, GUARD:GUARD + 1024],
                                  in_=feats_own[s * 64:(s + 1) * 64, :])

                OT = otp.tile([64, 1024], f32, tag="OT", name="OT")
                if do_conv:
                    H1 = xtp.tile([128, GW], f32r, tag="H1", name="H1")
                    nc.vector.memset(H1[:, 0:GUARD].bitcast(f32), 0.0)
                    nc.vector.memset(H1[:, GUARD + 1024:GW].bitcast(f32), 0.0)
                    conv_layer([P, F], [wa1_t, wb1_t], [128, 64], 128, b1_t,
                               H1, True)

                    H2 = xtp.tile([128, GW], f32r, tag="H2", name="H2")
                    nc.vector.memset(H2[:, 0:GUARD].bitcast(f32), 0.0)
                    nc.vector.memset(H2[:, GUARD + 1024:GW].bitcast(f32), 0.0)
                    conv_layer([H1], [w2p_t], [128], 128, b2_t, H2, True)

                    conv_layer([H2], [w3p_t], [128], 64, b3_t, OT, False)
                else:
                    nc.vector.tensor_copy(out=OT[:, :], in_=P[:64, 33:1057])
                nc.sync.dma_start(out=out_own[s * 64:(s + 1) * 64, :],
                                  in_=OT[:, :])
    nc.finalize()
    return nc


# ------------------------------------------------------------- entry point

def kernel(feats, edges, W1, b1, W2, b2, W3, b3):
    import sys
    if "/opt/trn_rl_repo" not in sys.path:
        sys.path.insert(0, "/opt/trn_rl_repo")
    from concourse.bass_utils import run_bass_kernel_spmd

    in_maps, node_lists, G, goff, Gtot = _host_prep(
        feats, edges, W1, b1, W2, b2, W3, b3)
    nc = _build_program(G, goff, Gtot)
    res = run_bass_kernel_spmd(nc, in_maps, core_ids=list(range(NCORES)))
    global LAST_EXEC_TIME_NS
    LAST_EXEC_TIME_NS = res.exec_time_ns
    out = np.zeros((V, C, H, W), np.float32)
    for c in range(NCORES):
        oo = np.asarray(res.results[c]["out_own"]).reshape(QPC, 64, 1024)
        for s in range(QPC):
            for n_local, v in enumerate(node_lists[c][s]):
                out[v] = oo[s, 16 * n_local:16 * n_local + 16].reshape(
                    16, 32, 32)
    return out



# revision 11
# speedup vs baseline: 12.4713x; 12.4713x over previous
"""Trainium2 Bass kernel for nn_CMP_3367254360436 (gnn_message_passing).

Reference computation: bidirectional signed scatter-add pooling over 8192
edges on 2048 nodes of [16,32,32] fp32 feature maps, concat [feats, pooled_pos,
pooled_neg] (48 ch), then three 3x3 SAME convs (48->32->32->16) with leaky
ReLU (0.1).

Device decomposition (per NeuronCore, 256 nodes/core in 64 quads of 4 nodes):
  1. Pooling: one dma_gather per quad pulls (contribution, channel) rows
     (idx = src_node*16 + ch, 4KB elements) from the full feats array into
     [128 rows, G, 1024] SBUF; compile-time 0/1 selection matrices S then
     accumulate rows into pooled (node, sign, ch) slots via fp32r matmuls in
     PSUM. ACT evacuates into a guarded tile P [128 = 4n x 32ch, 33+1024+33].
  2. Convs: 3x3 conv = 9 shifted flat-window matmuls with block-diagonal
     (4-node) weights accumulating in PSUM. Taps are split into 3 groups by
     kx so all matmuls of a PSUM accumulation group write identical bytes;
     the wrapped boundary column of groups kx=0 / kx=2 is zeroed afterwards
     with a strided memset. Groups are combined on DVE; bias + leaky ReLU on
     ACT (Lrelu) writes the next guarded tile.
  3. conv3 output [64 = 4n x 16ch, 1024] is DMA'd back to HBM.

The Bass program is identical on all 8 cores (SPMD); all per-core variation
(node assignment, S matrices, gather indices) is carried in the input data.
"""

import numpy as np

LAST_EXEC_TIME_NS = None
V, C, H, W = 2048, 16, 32, 32
NCORES = 8
NPQ = 4                      # nodes per quad
QPC = V // NCORES // NPQ     # quads per core = 64
GUARD = 33
GW = GUARD + 1024 + GUARD    # guarded tile free width = 1090

# v2 padded-row layout: each 32-px image row stored 33 wide with a shared
# zero column between rows, so kx=+/-1 tap shifts read zeros instead of
# wrapping into the neighbouring row. 9 taps then share one PSUM group.
PW = 33                      # padded row width
PINT = 32 * PW               # padded interior = 1056
PGUARD = 34                  # covers max |delta| = 33 + 1
PGW = PGUARD + PINT + PGUARD  # 1124
# conv chunk row split (rows of 33): matmul N = 396, 330, 330 (<=512 f32,
# all even -- fp32r matmuls require even innermost free-dim counts)
CHUNK_ROWS = (12, 10, 10)


# ---------------------------------------------------------------- host prep

def _host_prep(feats, edges, W1, b1, W2, b2, W3, b3):
    edges = np.asarray(edges).reshape(-1, 3)
    src, sign, dst = edges[:, 0], edges[:, 1], edges[:, 2]
    feats = np.ascontiguousarray(np.asarray(feats), dtype=np.float32)

    pos = [[] for _ in range(V)]
    neg = [[] for _ in range(V)]
    for s, sg, d in zip(src, sign, dst):
        buck = pos if sg > 0 else neg
        buck[int(d)].append(int(s))
        buck[int(s)].append(int(d))

    wgt = np.array([len(pos[v]) + len(neg[v]) for v in range(V)])

    # degree-balanced quads: snake-deal sorted nodes into NCORES*QPC quads,
    # then deal quads (sorted by weight) across cores per slot so per-slot
    # group counts line up across cores.
    order = np.argsort(-wgt, kind="stable")
    nquads = NCORES * QPC
    quads = [[] for _ in range(nquads)]
    for i, v in enumerate(order):
        r, j = divmod(i, nquads)
        q = j if r % 2 == 0 else nquads - 1 - j
        quads[q].append(int(v))
    qw = [sum(wgt[v] for v in q) for q in quads]
    qorder = np.argsort(-np.array(qw), kind="stable")
    assign = np.array(qorder).reshape(QPC, NCORES)  # [slot, core] -> quad id

    slot_rows = {}
    for s in range(QPC):
        for c in range(NCORES):
            rows = []
            for n_local, v in enumerate(quads[assign[s, c]]):
                for sgn, lst in ((0, pos[v]), (1, neg[v])):
                    for u in lst:
                        for ch in range(16):
                            rows.append((u * 16 + ch, 32 * n_local + 16 * sgn + ch))
            slot_rows[(c, s)] = rows
    G = np.zeros(QPC, dtype=np.int64)
    for s in range(QPC):
        G[s] = max(1, max((len(slot_rows[(c, s)]) + 127) // 128
                          for c in range(NCORES)))
    Gtot = int(G.sum())
    goff = np.concatenate([[0], np.cumsum(G)]).astype(np.int64)

    in_maps = []
    node_lists = []
    for c in range(NCORES):
        idxs_pack = np.zeros((16, 8 * Gtot), np.int16)   # idx j -> [j%16, j//16]
        S_pack = np.zeros((Gtot * 128, 128), np.float32)
        f_own = np.zeros((QPC * 64, 1024), np.float32)
        nodes_c = []
        for s in range(QPC):
            rows = slot_rows[(c, s)]
            base = int(goff[s])
            for j, (srcidx, slot) in enumerate(rows):
                jj = base * 128 + j
                idxs_pack[jj % 16, jj // 16] = srcidx
                S_pack[base * 128 + j, slot] = 1.0
            nodes = quads[assign[s, c]]
            nodes_c.append(nodes)
            for n_local, v in enumerate(nodes):
                f_own[s * 64 + 16 * n_local: s * 64 + 16 * n_local + 16] = \
                    feats[v].reshape(16, 1024)
        node_lists.append(nodes_c)
        # prepadded copy for the v2 padded-row layout (zeros at col 32 of
        # each 33-wide row, so the device never has to re-zero pad columns)
        f_own_p = np.zeros((QPC * 64, PINT), np.float32)
        f_own_p.reshape(-1, 32, PW)[:, :, :32] = f_own.reshape(-1, 32, 32)
        in_maps.append({
            "feats_all": feats.reshape(V * 16, 1024),
            "feats_own": f_own,
            "feats_own_p": f_own_p,
            # replicated across the 8 Q7 cores (16 partitions each)
            "idxs_pack": np.tile(idxs_pack, (8, 1)),
            "s_pack": S_pack,
        })

    # block-diag weight packs, stored as [K, 9*M] with tap t = 3*ky + kx
    W1 = np.asarray(W1); W2 = np.asarray(W2); W3 = np.asarray(W3)
    wa1 = np.zeros((128, 9, 128), np.float32)
    wb1 = np.zeros((64, 9, 128), np.float32)
    w2p = np.zeros((128, 9, 128), np.float32)
    w3p = np.zeros((128, 9, 64), np.float32)
    for ky in range(3):
        for kx in range(3):
            t = 3 * ky + kx
            for n in range(4):
                wa1[32*n:32*n+32, t, 32*n:32*n+32] = W1[:, 16:48, ky, kx].T
                wb1[16*n:16*n+16, t, 32*n:32*n+32] = W1[:, 0:16, ky, kx].T
                w2p[32*n:32*n+32, t, 32*n:32*n+32] = W2[:, :, ky, kx].T
                w3p[32*n:32*n+32, t, 16*n:16*n+16] = W3[:, :, ky, kx].T
    consts = {
        "wa1": wa1.reshape(128, 9 * 128), "wb1": wb1.reshape(64, 9 * 128),
        "w2p": w2p.reshape(128, 9 * 128), "w3p": w3p.reshape(128, 9 * 64),
        "b1t": np.tile(np.asarray(b1), 4).astype(np.float32).reshape(128, 1),
        "b2t": np.tile(np.asarray(b2), 4).astype(np.float32).reshape(128, 1),
        "b3t": np.tile(np.asarray(b3), 4).astype(np.float32).reshape(64, 1),
    }
    for m in in_maps:
        m.update({k: v.copy() for k, v in consts.items()})
    return in_maps, node_lists, G, goff, Gtot


# ------------------------------------------------------------- bass program

def _build_program(G, goff, Gtot, leaky_on_act=True, nslots=QPC,
                   for_sim=False, parts="all"):
    import concourse.mybir as mybir
    from concourse import bacc
    from concourse.tile import TileContext

    f32 = mybir.dt.float32
    f32r = mybir.dt.float32r
    nc = bacc.Bacc("TRN2", target_bir_lowering=False)

    feats_all = nc.dram_tensor("feats_all", [V * 16, 1024], f32r,
                               kind="ExternalInput")
    feats_own = nc.dram_tensor("feats_own", [QPC * 64, 1024], f32r,
                               kind="ExternalInput")
    idxs_pack = nc.dram_tensor("idxs_pack", [128, 8 * Gtot], mybir.dt.int16,
                               kind="ExternalInput")
    s_pack = nc.dram_tensor("s_pack", [Gtot * 128, 128], f32r,
                            kind="ExternalInput")
    wa1 = nc.dram_tensor("wa1", [128, 9 * 128], f32r, kind="ExternalInput")
    wb1 = nc.dram_tensor("wb1", [64, 9 * 128], f32r, kind="ExternalInput")
    w2p = nc.dram_tensor("w2p", [128, 9 * 128], f32r, kind="ExternalInput")
    w3p = nc.dram_tensor("w3p", [128, 9 * 64], f32r, kind="ExternalInput")
    b1t = nc.dram_tensor("b1t", [128, 1], f32, kind="ExternalInput")
    b2t = nc.dram_tensor("b2t", [128, 1], f32, kind="ExternalInput")
    b3t = nc.dram_tensor("b3t", [64, 1], f32, kind="ExternalInput")
    out_own = nc.dram_tensor("out_own", [QPC * 64, 1024], f32,
                             kind="ExternalOutput")

    # HW probe: Lrelu ignores the alpha operand (table slope 0.01);
    # Prelu honors alpha and matches leaky(0.1) exactly.
    LRELU = mybir.ActivationFunctionType.Prelu
    Gmax = int(G.max())


    with TileContext(nc) as tc:
        with (
            tc.tile_pool(name="const", bufs=1) as constp,
            tc.tile_pool(name="gath", bufs=3) as gathp,
            tc.tile_pool(name="stile", bufs=3) as stilep,
            tc.tile_pool(name="xt", bufs=3) as xtp,
            tc.tile_pool(name="comb", bufs=6) as combp,
            tc.tile_pool(name="otile", bufs=3) as otp,
            tc.tile_pool(name="poolps", bufs=1, space="PSUM") as poolpsp,
            tc.tile_pool(name="convps", bufs=6, space="PSUM") as convpsp,
        ):
            # ---- resident constants
            wa1_t = constp.tile([128, 9 * 128], f32r)
            wb1_t = constp.tile([64, 9 * 128], f32r)
            w2p_t = constp.tile([128, 9 * 128], f32r)
            w3p_t = constp.tile([128, 9 * 64], f32r)
            nc.sync.dma_start(out=wa1_t[:, :], in_=wa1[:, :])
            nc.sync.dma_start(out=wb1_t[:, :], in_=wb1[:, :])
            nc.sync.dma_start(out=w2p_t[:, :], in_=w2p[:, :])
            nc.sync.dma_start(out=w3p_t[:, :], in_=w3p[:, :])
            b1_t = constp.tile([128, 1], f32)
            b2_t = constp.tile([128, 1], f32)
            b3_t = constp.tile([64, 1], f32)
            nc.sync.dma_start(out=b1_t[:, :], in_=b1t[:, :])
            nc.sync.dma_start(out=b2_t[:, :], in_=b2t[:, :])
            nc.sync.dma_start(out=b3_t[:, :], in_=b3t[:, :])
            idx_t = constp.tile([128, 8 * Gtot], mybir.dt.int16)
            nc.sync.dma_start(out=idx_t[:, :], in_=idxs_pack[:, :])

            def conv_layer(x_tiles, w_tiles, Ks, M, bias, out_tile,
                           out_guarded):
                for w0 in (0, 512):
                    gps = []
                    for kx in range(3):
                        ps = convpsp.tile([128, 512], f32, tag="convps",
                                          name=f"ps_{kx}")
                        nmm = 3 * len(x_tiles)
                        i = 0
                        for ky in range(3):
                            t = 3 * ky + kx
                            delta = (ky - 1) * 32 + (kx - 1)
                            for xt, wt, K in zip(x_tiles, w_tiles, Ks):
                                a = GUARD + w0 + delta
                                nc.tensor.matmul(
                                    ps[:M, :],
                                    wt[:K, t * M:(t + 1) * M],
                                    xt[:K, a:a + 512],
                                    start=(i == 0), stop=(i == nmm - 1),
                                )
                                i += 1
                        # zero the wrapped boundary column
                        col = {0: 0, 2: 31}.get(kx)
                        if col is not None:
                            colap = ps[:M, :].rearrange(
                                "p (r c) -> p r c", c=32)[:, :, col:col + 1]
                            nc.vector.memset(colap, 0.0)
                        gps.append(ps)
                    s_t = combp.tile([128, 512], f32, tag="comb", name="s_t")
                    nc.vector.tensor_copy(out=s_t[:M, :], in_=gps[0][:M, :])
                    nc.vector.tensor_tensor(out=s_t[:M, :], in0=s_t[:M, :],
                                            in1=gps[1][:M, :],
                                            op=mybir.AluOpType.add)
                    nc.vector.tensor_tensor(out=s_t[:M, :], in0=s_t[:M, :],
                                            in1=gps[2][:M, :],
                                            op=mybir.AluOpType.add)
                    off = GUARD + w0 if out_guarded else w0
                    if leaky_on_act:
                        nc.scalar.activation(out_tile[:M, off:off + 512],
                                             s_t[:M, :], LRELU,
                                             bias=bias[:M, :], alpha=0.1)
                    else:
                        # leaky(x+b) = max(x+b, 0.1*(x+b)) on DVE
                        sb = combp.tile([128, 512], f32, tag="comb2",
                                        name="sb")
                        nc.vector.tensor_scalar(
                            out=sb[:M, :], in0=s_t[:M, :],
                            scalar1=bias[:M, :], scalar2=0.1,
                            op0=mybir.AluOpType.add,
                            op1=mybir.AluOpType.mult)
                        nc.vector.tensor_scalar(
                            out=s_t[:M, :], in0=s_t[:M, :],
                            scalar1=bias[:M, :], scalar2=None,
                            op0=mybir.AluOpType.add)
                        nc.vector.tensor_tensor(
                            out=out_tile[:M, off:off + 512], in0=s_t[:M, :],
                            in1=sb[:M, :], op=mybir.AluOpType.max)

            for s in range(nslots):
                g = int(G[s])
                base = int(goff[s])
                do_pool = parts in ("all", "pool")
                do_conv = parts in ("all", "conv")
                # ---- pooling gather: [128, g, 1024]
                if do_pool:
                  gath = gathp.tile([128, Gmax * 1024], f32r, tag="gath",
                                    name="gath")
                  nc.gpsimd.dma_gather(
                      out_ap=gath[:, :g * 1024].rearrange(
                          "p (gg f) -> p gg f", f=1024),
                      in_ap=feats_all[:, :],
                      idxs_ap=idx_t[:, base * 8:(base + g) * 8],
                      num_idxs=g * 128,
                      num_idxs_reg=g * 128,
                      elem_size=1024,
                      single_packet=False,
                  )
                  s_t = stilep.tile([128, Gmax * 128], f32r, tag="stile",
                                    name="s_mat")
                  nc.sync.dma_start(
                      out=s_t[:, :g * 128].rearrange("p (gg m) -> p gg m",
                                                     m=128),
                      in_=s_pack[base * 128:(base + g) * 128, :].rearrange(
                          "(gg p) m -> p gg m", p=128),
                  )
                  pool_ps = poolpsp.tile([128, 1024], f32, tag="poolps",
                                         name="pool_ps")
                  for w0 in (0, 512):
                      for gg in range(g):
                          nc.tensor.matmul(
                              pool_ps[:, w0:w0 + 512],
                              s_t[:, gg * 128:(gg + 1) * 128],
                              gath[:, gg * 1024 + w0:gg * 1024 + w0 + 512],
                              start=(gg == 0), stop=(gg == g - 1),
                          )
                # ---- guarded input tiles
                P = xtp.tile([128, GW], f32r, tag="P", name="P")
                nc.vector.memset(P[:, 0:GUARD].bitcast(f32), 0.0)
                nc.vector.memset(P[:, GUARD + 1024:GW].bitcast(f32), 0.0)
                if do_pool:
                    nc.vector.tensor_copy(out=P[:, GUARD:GUARD + 1024],
                                           in_=pool_ps[:, :])
                else:
                    nc.vector.memset(P[:, GUARD:GUARD + 1024].bitcast(f32),
                                     0.0)
                F = xtp.tile([64, GW], f32r, tag="F", name="F")
                nc.vector.memset(F[:, 0:GUARD].bitcast(f32), 0.0)
                nc.vector.memset(F[:, GUARD + 1024:GW].bitcast(f32), 0.0)
                nc.sync.dma_start(out=F[:, GUARD:GUARD + 1024],
                                  in_=feats_own[s * 64:(s + 1) * 64, :])

                OT = otp.tile([64, 1024], f32, tag="OT", name="OT")
                if do_conv:
                    H1 = xtp.tile([128, GW], f32r, tag="H1", name="H1")
                    nc.vector.memset(H1[:, 0:GUARD].bitcast(f32), 0.0)
                    nc.vector.memset(H1[:, GUARD + 1024:GW].bitcast(f32), 0.0)
                    conv_layer([P, F], [wa1_t, wb1_t], [128, 64], 128, b1_t,
                               H1, True)

                    H2 = xtp.tile([128, GW], f32r, tag="H2", name="H2")
                    nc.vector.memset(H2[:, 0:GUARD].bitcast(f32), 0.0)
                    nc.vector.memset(H2[:, GUARD + 1024:GW].bitcast(f32), 0.0)
                    conv_layer([H1], [w2p_t], [128], 128, b2_t, H2, True)

                    conv_layer([H2], [w3p_t], [128], 64, b3_t, OT, False)
                else:
                    nc.vector.tensor_copy(out=OT[:, :], in_=P[:64, 33:1057])
                nc.sync.dma_start(out=out_own[s * 64:(s + 1) * 64, :],
                                  in_=OT[:, :])
    nc.finalize()
    return nc


# ------------------------------------------------------- bass program (v2)

def _build_program_v2(G, goff, Gtot, nslots=QPC, parts="all"):
    """Padded-row layout: all 9 taps of a conv accumulate into ONE PSUM
    group (inputs pre-shifted via the shared zero column between 33-wide
    rows), so the per-kx PSUM split + DVE combine of v1 disappears.
    Guard/pad zeros live in persistent ring tiles zeroed once at start.
    Gathers are batched 2 slots per dma_gather, S loads 4 slots per DMA,
    outputs 2 slots per DMA."""
    import concourse.mybir as mybir
    from concourse import bacc
    from concourse.tile import TileContext

    f32 = mybir.dt.float32
    f32r = mybir.dt.float32r
    nc = bacc.Bacc("TRN2", target_bir_lowering=False)

    feats_all = nc.dram_tensor("feats_all", [V * 16, 1024], f32r,
                               kind="ExternalInput")
    feats_own_p = nc.dram_tensor("feats_own_p", [QPC * 64, PINT], f32r,
                                 kind="ExternalInput")
    idxs_pack = nc.dram_tensor("idxs_pack", [128, 8 * Gtot], mybir.dt.int16,
                               kind="ExternalInput")
    s_pack = nc.dram_tensor("s_pack", [Gtot * 128, 128], f32r,
                            kind="ExternalInput")
    wa1 = nc.dram_tensor("wa1", [128, 9 * 128], f32r, kind="ExternalInput")
    wb1 = nc.dram_tensor("wb1", [64, 9 * 128], f32r, kind="ExternalInput")
    w2p = nc.dram_tensor("w2p", [128, 9 * 128], f32r, kind="ExternalInput")
    w3p = nc.dram_tensor("w3p", [128, 9 * 64], f32r, kind="ExternalInput")
    b1t = nc.dram_tensor("b1t", [128, 1], f32, kind="ExternalInput")
    b2t = nc.dram_tensor("b2t", [128, 1], f32, kind="ExternalInput")
    b3t = nc.dram_tensor("b3t", [64, 1], f32, kind="ExternalInput")
    out_own = nc.dram_tensor("out_own", [QPC * 64, 1024], f32,
                             kind="ExternalOutput")

    LRELU = mybir.ActivationFunctionType.Prelu

    # batch extents
    gb_max = max((int(G[b: b + 2].sum()) for b in range(0, nslots, 2)),
                 default=1)
    sb_max = max((int(G[b: b + 4].sum()) for b in range(0, nslots, 4)),
                 default=1)

    with TileContext(nc) as tc:
        with (
            tc.tile_pool(name="const", bufs=1) as constp,
            tc.tile_pool(name="gath", bufs=2) as gathp,
            tc.tile_pool(name="stile", bufs=2) as stilep,
            tc.tile_pool(name="poolps", bufs=2, space="PSUM") as poolpsp,
            tc.tile_pool(name="convps", bufs=4, space="PSUM") as convpsp,
        ):
            # ---- resident constants
            wa1_t = constp.tile([128, 9 * 128], f32r)
            wb1_t = constp.tile([64, 9 * 128], f32r)
            w2p_t = constp.tile([128, 9 * 128], f32r)
            w3p_t = constp.tile([128, 9 * 64], f32r)
            nc.sync.dma_start(out=wa1_t[:, :], in_=wa1[:, :])
            nc.sync.dma_start(out=wb1_t[:, :], in_=wb1[:, :])
            nc.sync.dma_start(out=w2p_t[:, :], in_=w2p[:, :])
            nc.sync.dma_start(out=w3p_t[:, :], in_=w3p[:, :])
            b1_t = constp.tile([128, 1], f32)
            b2_t = constp.tile([128, 1], f32)
            b3_t = constp.tile([64, 1], f32)
            nc.sync.dma_start(out=b1_t[:, :], in_=b1t[:, :])
            nc.sync.dma_start(out=b2_t[:, :], in_=b2t[:, :])
            nc.sync.dma_start(out=b3_t[:, :], in_=b3t[:, :])
            idx_t = constp.tile([128, 8 * Gtot], mybir.dt.int16)
            nc.sync.dma_start(out=idx_t[:, :], in_=idxs_pack[:, :])

            # ---- persistent ring tiles; guards + pad columns zeroed once
            def ring(name, p, w, n=3):
                ts = [constp.tile([p, w], f32r, name=f"{name}{i}")
                      for i in range(n)]
                for t in ts:
                    nc.vector.memset(t[:, :].bitcast(f32), 0.0)
                return ts

            P_ring = ring("Pr", 128, PGW)
            F_ring = ring("Fr", 64, PGW)
            H1_ring = ring("H1r", 128, PGW)
            H2_ring = ring("H2r", 128, PGW)
            OT_ring = [constp.tile([128, 1024], f32, name=f"OTr{i}")
                       for i in range(3)]

            def conv_layer(x_tiles, w_tiles, Ks, M, bias, out_tile,
                           out_padded, out_rowbase=0):
                r0 = 0
                for rows in CHUNK_ROWS:
                    Nc = rows * PW
                    ps = convpsp.tile([128, 512], f32, tag="convps",
                                      name="cps")
                    nmm = 9 * len(x_tiles)
                    i = 0
                    for ky in range(3):
                        for kx in range(3):
                            t = 3 * ky + kx
                            delta = PW * (ky - 1) + (kx - 1)
                            a = PGUARD + PW * r0 + delta
                            for xt, wt, K in zip(x_tiles, w_tiles, Ks):
                                nc.tensor.matmul(
                                    ps[:M, :Nc],
                                    wt[:K, t * M:(t + 1) * M],
                                    xt[:K, a:a + Nc],
                                    start=(i == 0), stop=(i == nmm - 1),
                                )
                                i += 1
                    ps_ap = ps[:M, :Nc].rearrange(
                        "p (r c) -> p r c", c=PW)[:, :, 0:32]
                    if out_padded:
                        off = PGUARD + PW * r0
                        out_ap = out_tile[:M, off:off + Nc].rearrange(
                            "p (r c) -> p r c", c=PW)[:, :, 0:32]
                    else:
                        out_ap = out_tile[
                            out_rowbase:out_rowbase + M,
                            32 * r0:32 * (r0 + rows)].rearrange(
                                "p (r c) -> p r c", c=32)
                    nc.scalar.activation(out_ap, ps_ap, LRELU,
                                         bias=bias[:M, :], alpha=0.1)
                    r0 += rows

            _state = {}
            for s in range(nslots):
                g = int(G[s])
                do_pool = parts in ("all", "pool")
                do_conv = parts in ("all", "conv")
                P = P_ring[s % 3]
                F = F_ring[s % 3]
                OT = OT_ring[(s // 2) % 3]
                ot_base = 64 * (s % 2)

                if do_pool:
                    if s % 2 == 0:
                        gsum = int(G[s: s + 2].sum())
                        base = int(goff[s])
                        gath = gathp.tile([128, gb_max * 1024], f32r,
                                          tag="gath", name="gath")
                        nc.gpsimd.dma_gather(
                            out_ap=gath[:, :gsum * 1024].rearrange(
                                "p (gg f) -> p gg f", f=1024),
                            in_ap=feats_all[:, :],
                            idxs_ap=idx_t[:, base * 8:(base + gsum) * 8],
                            num_idxs=gsum * 128,
                            num_idxs_reg=gsum * 128,
                            elem_size=1024,
                            single_packet=False,
                        )
                        _state["gath"] = gath
                    if s % 4 == 0:
                        ssum = int(G[s: s + 4].sum())
                        sbase = int(goff[s])
                        s_t = stilep.tile([128, sb_max * 128], f32r,
                                          tag="stile", name="s_mat")
                        nc.sync.dma_start(
                            out=s_t[:, :ssum * 128].rearrange(
                                "p (gg m) -> p gg m", m=128),
                            in_=s_pack[
                                sbase * 128:(sbase + ssum) * 128,
                                :].rearrange("(gg p) m -> p gg m", p=128),
                        )
                        _state["s_t"] = s_t
                    gath = _state["gath"]
                    s_t = _state["s_t"]
                    boff = int(goff[s]) - int(goff[2 * (s // 2)])
                    soff = int(goff[s]) - int(goff[4 * (s // 4)])
                    pool_ps = poolpsp.tile([128, 1024], f32, tag="poolps",
                                           name="pool_ps")
                    for w0 in (0, 512):
                        for gg in range(g):
                            nc.tensor.matmul(
                                pool_ps[:, w0:w0 + 512],
                                s_t[:, (soff + gg) * 128:
                                    (soff + gg + 1) * 128],
                                gath[:, (boff + gg) * 1024 + w0:
                                     (boff + gg) * 1024 + w0 + 512],
                                start=(gg == 0), stop=(gg == g - 1),
                            )
                    # evacuate compact pool PSUM into padded P interior
                    nc.vector.tensor_copy(
                        out=P[:, PGUARD:PGUARD + PINT].rearrange(
                            "p (r c) -> p r c", c=PW)[:, :, 0:32],
                        in_=pool_ps[:, :].rearrange(
                            "p (r c) -> p r c", c=32),
                    )

                nc.sync.dma_start(
                    out=F[:, PGUARD:PGUARD + PINT],
                    in_=feats_own_p[s * 64:(s + 1) * 64, :])

                if do_conv:
                    H1 = H1_ring[s % 3]
                    H2 = H2_ring[s % 3]
                    conv_layer([P, F], [wa1_t, wb1_t], [128, 64], 128,
                               b1_t, H1, True)
                    conv_layer([H1], [w2p_t], [128], 128, b2_t, H2, True)
                    conv_layer([H2], [w3p_t], [128], 64, b3_t, OT, False,
                               out_rowbase=ot_base)
                elif do_pool:
                    nc.vector.tensor_copy(
                        out=OT[ot_base:ot_base + 64, :],
                        in_=pool_ps[:64, :])
                else:
                    nc.vector.memset(OT[ot_base:ot_base + 64, :], 0.0)

                if s % 2 == 1 or s == nslots - 1:
                    nrows = ot_base + 64
                    row0 = (s // 2) * 128
                    nc.sync.dma_start(
                        out=out_own[row0:row0 + nrows, :],
                        in_=OT[:nrows, :])
    nc.finalize()
    return nc


# ------------------------------------------------------------- entry point

def kernel(feats, edges, W1, b1, W2, b2, W3, b3):
    import sys
    if "/opt/trn_rl_repo" not in sys.path:
        sys.path.insert(0, "/opt/trn_rl_repo")
    from concourse.bass_utils import run_bass_kernel_spmd

    in_maps, node_lists, G, goff, Gtot = _host_prep(
        feats, edges, W1, b1, W2, b2, W3, b3)
    nc = _build_program_v2(G, goff, Gtot)
    try:
        res = run_bass_kernel_spmd(nc, in_maps, core_ids=list(range(NCORES)))
    except Exception:
        # transient axon tunnel failures (e.g. "mesh desynced") are
        # recoverable on a fresh dispatch
        res = run_bass_kernel_spmd(nc, in_maps, core_ids=list(range(NCORES)))
    global LAST_EXEC_TIME_NS
    LAST_EXEC_TIME_NS = res.exec_time_ns
    out = np.zeros((V, C, H, W), np.float32)
    for c in range(NCORES):
        oo = np.asarray(res.results[c]["out_own"]).reshape(QPC, 64, 1024)
        for s in range(QPC):
            for n_local, v in enumerate(node_lists[c][s]):
                out[v] = oo[s, 16 * n_local:16 * n_local + 16].reshape(
                    16, 32, 32)
    return out



# revision 22
# speedup vs baseline: 17.4347x; 1.3980x over previous
"""Trainium2 Bass kernel for nn_CMP_3367254360436 (gnn_message_passing).

Reference computation: bidirectional signed scatter-add pooling over 8192
edges on 2048 nodes of [16,32,32] fp32 feature maps, concat [feats, pooled_pos,
pooled_neg] (48 ch), then three 3x3 SAME convs (48->32->32->16) with leaky
ReLU (0.1).

Device decomposition (per NeuronCore, 256 nodes/core in 64 quads of 4 nodes),
v2 = _build_program_v2, the active builder:
  1. Pooling: one dma_gather per 2 quads pulls (contribution, channel) rows
     (idx = src_node*16 + ch, 4KB elements) from the full feats array into
     [128 rows, G, 1024] SBUF; compile-time 0/1 selection matrices S (loaded
     4 quads per DMA) accumulate rows into pooled (node, sign, ch) slots via
     fp32r matmuls in PSUM.
  2. Padded-row image layout: each 32-px row is stored 33 wide with a shared
     zero column between rows, inside persistent ring tiles
     [128 = 4n x 32ch, 34+32*33+34] whose guard/pad zeros are written ONCE
     at program start (interiors are fully rewritten every iteration, pad
     columns are skipped by strided writes). kx=+/-1 tap shifts then read
     zeros instead of wrapping into the neighbouring row, so ALL 9 taps of a
     3x3 conv accumulate into a single PSUM group (identical byte ranges via
     pre-shifted input windows) -- no per-kx PSUM split, no DVE combine, no
     boundary fixups. PSUM chunking: rows split (12,10,10) -> matmul
     N = 396/330/330 (fp32r needs even N). ACT applies bias + leaky ReLU
     (Prelu, alpha=0.1) on evacuation.
  3. conv3 output lands compact in a [128, 1024] tile (2 quads), DMA'd back
     to HBM once per 2 quads.
Sim (CoreSim cost model): span 1.154 ms/core, PE 97.8% busy -- the PE cycle
count is the algorithmic floor for 4-node block-diagonal weight packing
(ceil(192/128)+1+1 K-groups x 9 taps x 1056 columns per quad + pooling
rows/128 x 1024).

The Bass program is identical on all 8 cores (SPMD); all per-core variation
(node assignment, S matrices, gather indices) is carried in the input data.
v1 (_build_program) is kept for reference: 3 kx PSUM groups combined on DVE
made DVE the co-bottleneck (sim 1.249 ms, PE 88%, DVE 79%).
"""

import numpy as np

LAST_EXEC_TIME_NS = None
V, C, H, W = 2048, 16, 32, 32
NCORES = 8
NPQ = 4                      # nodes per quad
QPC = V // NCORES // NPQ     # quads per core = 64
GUARD = 33
GW = GUARD + 1024 + GUARD    # guarded tile free width = 1090

# v2 padded-row layout: each 32-px image row stored 33 wide with a shared
# zero column between rows, so kx=+/-1 tap shifts read zeros instead of
# wrapping into the neighbouring row. 9 taps then share one PSUM group.
PW = 33                      # padded row width
PINT = 32 * PW               # padded interior = 1056
PGUARD = 34                  # covers max |delta| = 33 + 1
PGW = PGUARD + PINT + PGUARD  # 1124
# conv chunk row split (rows of 33): matmul N = 396, 330, 330 (<=512 f32,
# all even -- fp32r matmuls require even innermost free-dim counts)
CHUNK_ROWS = (12, 10, 10)


# ---------------------------------------------------------------- host prep

def _host_prep(feats, edges, W1, b1, W2, b2, W3, b3):
    edges = np.asarray(edges).reshape(-1, 3)
    src, sign, dst = edges[:, 0], edges[:, 1], edges[:, 2]
    feats = np.ascontiguousarray(np.asarray(feats), dtype=np.float32)

    pos = [[] for _ in range(V)]
    neg = [[] for _ in range(V)]
    for s, sg, d in zip(src, sign, dst):
        buck = pos if sg > 0 else neg
        buck[int(d)].append(int(s))
        buck[int(s)].append(int(d))

    wgt = np.array([len(pos[v]) + len(neg[v]) for v in range(V)])

    # degree-balanced quads: snake-deal sorted nodes into NCORES*QPC quads,
    # then deal quads (sorted by weight) across cores per slot so per-slot
    # group counts line up across cores.
    order = np.argsort(-wgt, kind="stable")
    nquads = NCORES * QPC
    quads = [[] for _ in range(nquads)]
    for i, v in enumerate(order):
        r, j = divmod(i, nquads)
        q = j if r % 2 == 0 else nquads - 1 - j
        quads[q].append(int(v))
    qw = [sum(wgt[v] for v in q) for q in quads]
    qorder = np.argsort(-np.array(qw), kind="stable")
    assign = np.array(qorder).reshape(QPC, NCORES)  # [slot, core] -> quad id

    slot_rows = {}
    for s in range(QPC):
        for c in range(NCORES):
            rows = []
            for n_local, v in enumerate(quads[assign[s, c]]):
                for sgn, lst in ((0, pos[v]), (1, neg[v])):
                    for u in lst:
                        for ch in range(16):
                            rows.append((u * 16 + ch, 32 * n_local + 16 * sgn + ch))
            slot_rows[(c, s)] = rows
    G = np.zeros(QPC, dtype=np.int64)
    for s in range(QPC):
        G[s] = max(1, max((len(slot_rows[(c, s)]) + 127) // 128
                          for c in range(NCORES)))
    Gtot = int(G.sum())
    goff = np.concatenate([[0], np.cumsum(G)]).astype(np.int64)

    in_maps = []
    node_lists = []
    for c in range(NCORES):
        idxs_pack = np.zeros((16, 8 * Gtot), np.int16)   # idx j -> [j%16, j//16]
        S_pack = np.zeros((Gtot * 128, 128), np.float32)
        f_own = np.zeros((QPC * 64, 1024), np.float32)
        nodes_c = []
        for s in range(QPC):
            rows = slot_rows[(c, s)]
            base = int(goff[s])
            for j, (srcidx, slot) in enumerate(rows):
                jj = base * 128 + j
                idxs_pack[jj % 16, jj // 16] = srcidx
                S_pack[base * 128 + j, slot] = 1.0
            nodes = quads[assign[s, c]]
            nodes_c.append(nodes)
            for n_local, v in enumerate(nodes):
                f_own[s * 64 + 16 * n_local: s * 64 + 16 * n_local + 16] = \
                    feats[v].reshape(16, 1024)
        node_lists.append(nodes_c)
        # prepadded copy for the v2 padded-row layout (zeros at col 32 of
        # each 33-wide row, so the device never has to re-zero pad columns)
        f_own_p = np.zeros((QPC * 64, PINT), np.float32)
        f_own_p.reshape(-1, 32, PW)[:, :, :32] = f_own.reshape(-1, 32, 32)
        in_maps.append({
            "feats_all": feats.reshape(V * 16, 1024),
            "feats_own": f_own,
            "feats_own_p": f_own_p,
            # replicated across the 8 Q7 cores (16 partitions each)
            "idxs_pack": np.tile(idxs_pack, (8, 1)),
            "s_pack": S_pack,
        })

    # block-diag weight packs, stored as [K, 9*M] with tap t = 3*ky + kx
    W1 = np.asarray(W1); W2 = np.asarray(W2); W3 = np.asarray(W3)
    wa1 = np.zeros((128, 9, 128), np.float32)
    wb1 = np.zeros((64, 9, 128), np.float32)
    w2p = np.zeros((128, 9, 128), np.float32)
    w3p = np.zeros((128, 9, 64), np.float32)
    for ky in range(3):
        for kx in range(3):
            t = 3 * ky + kx
            for n in range(4):
                wa1[32*n:32*n+32, t, 32*n:32*n+32] = W1[:, 16:48, ky, kx].T
                wb1[16*n:16*n+16, t, 32*n:32*n+32] = W1[:, 0:16, ky, kx].T
                w2p[32*n:32*n+32, t, 32*n:32*n+32] = W2[:, :, ky, kx].T
                w3p[32*n:32*n+32, t, 16*n:16*n+16] = W3[:, :, ky, kx].T
    consts = {
        "wa1": wa1.reshape(128, 9 * 128), "wb1": wb1.reshape(64, 9 * 128),
        "w2p": w2p.reshape(128, 9 * 128), "w3p": w3p.reshape(128, 9 * 64),
        "b1t": np.tile(np.asarray(b1), 4).astype(np.float32).reshape(128, 1),
        "b2t": np.tile(np.asarray(b2), 4).astype(np.float32).reshape(128, 1),
        "b3t": np.tile(np.asarray(b3), 4).astype(np.float32).reshape(64, 1),
    }
    for m in in_maps:
        m.update({k: v.copy() for k, v in consts.items()})
        # bf16 twins for the half-precision program variant (suffix _h);
        # matmul operands only -- biases and the output stay f32
        import ml_dtypes
        bf16 = ml_dtypes.bfloat16
        for k in ("feats_all", "feats_own_p", "s_pack",
                  "wa1", "wb1", "w2p", "w3p"):
            m[k + "_h"] = m[k].astype(bf16)
    return in_maps, node_lists, G, goff, Gtot


# ------------------------------------------------------------- bass program

def _build_program(G, goff, Gtot, leaky_on_act=True, nslots=QPC,
                   for_sim=False, parts="all"):
    import concourse.mybir as mybir
    from concourse import bacc
    from concourse.tile import TileContext

    f32 = mybir.dt.float32
    f32r = mybir.dt.float32r
    nc = bacc.Bacc("TRN2", target_bir_lowering=False)

    feats_all = nc.dram_tensor("feats_all", [V * 16, 1024], f32r,
                               kind="ExternalInput")
    feats_own = nc.dram_tensor("feats_own", [QPC * 64, 1024], f32r,
                               kind="ExternalInput")
    idxs_pack = nc.dram_tensor("idxs_pack", [128, 8 * Gtot], mybir.dt.int16,
                               kind="ExternalInput")
    s_pack = nc.dram_tensor("s_pack", [Gtot * 128, 128], f32r,
                            kind="ExternalInput")
    wa1 = nc.dram_tensor("wa1", [128, 9 * 128], f32r, kind="ExternalInput")
    wb1 = nc.dram_tensor("wb1", [64, 9 * 128], f32r, kind="ExternalInput")
    w2p = nc.dram_tensor("w2p", [128, 9 * 128], f32r, kind="ExternalInput")
    w3p = nc.dram_tensor("w3p", [128, 9 * 64], f32r, kind="ExternalInput")
    b1t = nc.dram_tensor("b1t", [128, 1], f32, kind="ExternalInput")
    b2t = nc.dram_tensor("b2t", [128, 1], f32, kind="ExternalInput")
    b3t = nc.dram_tensor("b3t", [64, 1], f32, kind="ExternalInput")
    out_own = nc.dram_tensor("out_own", [QPC * 64, 1024], f32,
                             kind="ExternalOutput")

    # HW probe: Lrelu ignores the alpha operand (table slope 0.01);
    # Prelu honors alpha and matches leaky(0.1) exactly.
    LRELU = mybir.ActivationFunctionType.Prelu
    Gmax = int(G.max())


    with TileContext(nc) as tc:
        with (
            tc.tile_pool(name="const", bufs=1) as constp,
            tc.tile_pool(name="gath", bufs=3) as gathp,
            tc.tile_pool(name="stile", bufs=3) as stilep,
            tc.tile_pool(name="xt", bufs=3) as xtp,
            tc.tile_pool(name="comb", bufs=6) as combp,
            tc.tile_pool(name="otile", bufs=3) as otp,
            tc.tile_pool(name="poolps", bufs=1, space="PSUM") as poolpsp,
            tc.tile_pool(name="convps", bufs=6, space="PSUM") as convpsp,
        ):
            # ---- resident constants
            wa1_t = constp.tile([128, 9 * 128], f32r)
            wb1_t = constp.tile([64, 9 * 128], f32r)
            w2p_t = constp.tile([128, 9 * 128], f32r)
            w3p_t = constp.tile([128, 9 * 64], f32r)
            nc.sync.dma_start(out=wa1_t[:, :], in_=wa1[:, :])
            nc.sync.dma_start(out=wb1_t[:, :], in_=wb1[:, :])
            nc.sync.dma_start(out=w2p_t[:, :], in_=w2p[:, :])
            nc.sync.dma_start(out=w3p_t[:, :], in_=w3p[:, :])
            b1_t = constp.tile([128, 1], f32)
            b2_t = constp.tile([128, 1], f32)
            b3_t = constp.tile([64, 1], f32)
            nc.sync.dma_start(out=b1_t[:, :], in_=b1t[:, :])
            nc.sync.dma_start(out=b2_t[:, :], in_=b2t[:, :])
            nc.sync.dma_start(out=b3_t[:, :], in_=b3t[:, :])
            idx_t = constp.tile([128, 8 * Gtot], mybir.dt.int16)
            nc.sync.dma_start(out=idx_t[:, :], in_=idxs_pack[:, :])

            def conv_layer(x_tiles, w_tiles, Ks, M, bias, out_tile,
                           out_guarded):
                for w0 in (0, 512):
                    gps = []
                    for kx in range(3):
                        ps = convpsp.tile([128, 512], f32, tag="convps",
                                          name=f"ps_{kx}")
                        nmm = 3 * len(x_tiles)
                        i = 0
                        for ky in range(3):
                            t = 3 * ky + kx
                            delta = (ky - 1) * 32 + (kx - 1)
                            for xt, wt, K in zip(x_tiles, w_tiles, Ks):
                                a = GUARD + w0 + delta
                                nc.tensor.matmul(
                                    ps[:M, :],
                                    wt[:K, t * M:(t + 1) * M],
                                    xt[:K, a:a + 512],
                                    start=(i == 0), stop=(i == nmm - 1),
                                )
                                i += 1
                        # zero the wrapped boundary column
                        col = {0: 0, 2: 31}.get(kx)
                        if col is not None:
                            colap = ps[:M, :].rearrange(
                                "p (r c) -> p r c", c=32)[:, :, col:col + 1]
                            nc.vector.memset(colap, 0.0)
                        gps.append(ps)
                    s_t = combp.tile([128, 512], f32, tag="comb", name="s_t")
                    nc.vector.tensor_copy(out=s_t[:M, :], in_=gps[0][:M, :])
                    nc.vector.tensor_tensor(out=s_t[:M, :], in0=s_t[:M, :],
                                            in1=gps[1][:M, :],
                                            op=mybir.AluOpType.add)
                    nc.vector.tensor_tensor(out=s_t[:M, :], in0=s_t[:M, :],
                                            in1=gps[2][:M, :],
                                            op=mybir.AluOpType.add)
                    off = GUARD + w0 if out_guarded else w0
                    if leaky_on_act:
                        nc.scalar.activation(out_tile[:M, off:off + 512],
                                             s_t[:M, :], LRELU,
                                             bias=bias[:M, :], alpha=0.1)
                    else:
                        # leaky(x+b) = max(x+b, 0.1*(x+b)) on DVE
                        sb = combp.tile([128, 512], f32, tag="comb2",
                                        name="sb")
                        nc.vector.tensor_scalar(
                            out=sb[:M, :], in0=s_t[:M, :],
                            scalar1=bias[:M, :], scalar2=0.1,
                            op0=mybir.AluOpType.add,
                            op1=mybir.AluOpType.mult)
                        nc.vector.tensor_scalar(
                            out=s_t[:M, :], in0=s_t[:M, :],
                            scalar1=bias[:M, :], scalar2=None,
                            op0=mybir.AluOpType.add)
                        nc.vector.tensor_tensor(
                            out=out_tile[:M, off:off + 512], in0=s_t[:M, :],
                            in1=sb[:M, :], op=mybir.AluOpType.max)

            for s in range(nslots):
                g = int(G[s])
                base = int(goff[s])
                do_pool = parts in ("all", "pool")
                do_conv = parts in ("all", "conv")
                # ---- pooling gather: [128, g, 1024]
                if do_pool:
                  gath = gathp.tile([128, Gmax * 1024], f32r, tag="gath",
                                    name="gath")
                  nc.gpsimd.dma_gather(
                      out_ap=gath[:, :g * 1024].rearrange(
                          "p (gg f) -> p gg f", f=1024),
                      in_ap=feats_all[:, :],
                      idxs_ap=idx_t[:, base * 8:(base + g) * 8],
                      num_idxs=g * 128,
                      num_idxs_reg=g * 128,
                      elem_size=1024,
                      single_packet=False,
                  )
                  s_t = stilep.tile([128, Gmax * 128], f32r, tag="stile",
                                    name="s_mat")
                  nc.sync.dma_start(
                      out=s_t[:, :g * 128].rearrange("p (gg m) -> p gg m",
                                                     m=128),
                      in_=s_pack[base * 128:(base + g) * 128, :].rearrange(
                          "(gg p) m -> p gg m", p=128),
                  )
                  pool_ps = poolpsp.tile([128, 1024], f32, tag="poolps",
                                         name="pool_ps")
                  for w0 in (0, 512):
                      for gg in range(g):
                          nc.tensor.matmul(
                              pool_ps[:, w0:w0 + 512],
                              s_t[:, gg * 128:(gg + 1) * 128],
                              gath[:, gg * 1024 + w0:gg * 1024 + w0 + 512],
                              start=(gg == 0), stop=(gg == g - 1),
                          )
                # ---- guarded input tiles
                P = xtp.tile([128, GW], f32r, tag="P", name="P")
                nc.vector.memset(P[:, 0:GUARD].bitcast(f32), 0.0)
                nc.vector.memset(P[:, GUARD + 1024:GW].bitcast(f32), 0.0)
                if do_pool:
                    nc.vector.tensor_copy(out=P[:, GUARD:GUARD + 1024],
                                           in_=pool_ps[:, :])
                else:
                    nc.vector.memset(P[:, GUARD:GUARD + 1024].bitcast(f32),
                                     0.0)
                F = xtp.tile([64, GW], f32r, tag="F", name="F")
                nc.vector.memset(F[:, 0:GUARD].bitcast(f32), 0.0)
                nc.vector.memset(F[:, GUARD + 1024:GW].bitcast(f32), 0.0)
                nc.sync.dma_start(out=F[:, GUARD:GUARD + 1024],
                                  in_=feats_own[s * 64:(s + 1) * 64, :])

                OT = otp.tile([64, 1024], f32, tag="OT", name="OT")
                if do_conv:
                    H1 = xtp.tile([128, GW], f32r, tag="H1", name="H1")
                    nc.vector.memset(H1[:, 0:GUARD].bitcast(f32), 0.0)
                    nc.vector.memset(H1[:, GUARD + 1024:GW].bitcast(f32), 0.0)
                    conv_layer([P, F], [wa1_t, wb1_t], [128, 64], 128, b1_t,
                               H1, True)

                    H2 = xtp.tile([128, GW], f32r, tag="H2", name="H2")
                    nc.vector.memset(H2[:, 0:GUARD].bitcast(f32), 0.0)
                    nc.vector.memset(H2[:, GUARD + 1024:GW].bitcast(f32), 0.0)
                    conv_layer([H1], [w2p_t], [128], 128, b2_t, H2, True)

                    conv_layer([H2], [w3p_t], [128], 64, b3_t, OT, False)
                else:
                    nc.vector.tensor_copy(out=OT[:, :], in_=P[:64, 33:1057])
                nc.sync.dma_start(out=out_own[s * 64:(s + 1) * 64, :],
                                  in_=OT[:, :])
    nc.finalize()
    return nc


# ------------------------------------------------------- bass program (v2)

def _build_program_v2(G, goff, Gtot, nslots=QPC, parts="all", half=True):
    """Padded-row layout: all 9 taps of a conv accumulate into ONE PSUM
    group (inputs pre-shifted via the shared zero column between 33-wide
    rows), so the per-kx PSUM split + DVE combine of v1 disappears.
    Guard/pad zeros live in persistent ring tiles zeroed once at start.
    Gathers are batched 2 slots per dma_gather, S loads 4 slots per DMA,
    outputs 2 slots per DMA. half=True uses bf16 matmul operands: fp32r
    matmuls pay a serial ~128-cycle weight load each (FWL is disabled for
    fp32), bf16 enables FWL so back-to-back matmuls stream at ~N cycles;
    PSUM accumulation stays f32 and the 2e-2 tolerance leaves ~3x margin
    over the ~7e-3 bf16 quantization error."""
    import concourse.mybir as mybir
    from concourse import bacc
    from concourse.tile import TileContext

    f32 = mybir.dt.float32
    f32r = mybir.dt.float32r
    mdt = mybir.dt.bfloat16 if half else f32r
    sfx = "_h" if half else ""
    nc = bacc.Bacc("TRN2", target_bir_lowering=False)

    feats_all = nc.dram_tensor("feats_all" + sfx, [V * 16, 1024], mdt,
                               kind="ExternalInput")
    feats_own_p = nc.dram_tensor("feats_own_p" + sfx, [QPC * 64, PINT], mdt,
                                 kind="ExternalInput")
    idxs_pack = nc.dram_tensor("idxs_pack", [128, 8 * Gtot], mybir.dt.int16,
                               kind="ExternalInput")
    s_pack = nc.dram_tensor("s_pack" + sfx, [Gtot * 128, 128], mdt,
                            kind="ExternalInput")
    wa1 = nc.dram_tensor("wa1" + sfx, [128, 9 * 128], mdt,
                         kind="ExternalInput")
    wb1 = nc.dram_tensor("wb1" + sfx, [64, 9 * 128], mdt,
                         kind="ExternalInput")
    w2p = nc.dram_tensor("w2p" + sfx, [128, 9 * 128], mdt,
                         kind="ExternalInput")
    w3p = nc.dram_tensor("w3p" + sfx, [128, 9 * 64], mdt,
                         kind="ExternalInput")
    b1t = nc.dram_tensor("b1t", [128, 1], f32, kind="ExternalInput")
    b2t = nc.dram_tensor("b2t", [128, 1], f32, kind="ExternalInput")
    b3t = nc.dram_tensor("b3t", [64, 1], f32, kind="ExternalInput")
    # output stays in the padded-row layout (host strips the pad columns);
    # keeps every ACT evacuation and the output DMA fully contiguous
    out_own = nc.dram_tensor("out_own", [QPC * 64, PINT], f32,
                             kind="ExternalOutput")

    LRELU = mybir.ActivationFunctionType.Prelu

    # batch extents
    gb_max = max((int(G[b: b + 2].sum()) for b in range(0, nslots, 2)),
                 default=1)
    sb_max = max((int(G[b: b + 4].sum()) for b in range(0, nslots, 4)),
                 default=1)

    with TileContext(nc) as tc:
        with (
            tc.tile_pool(name="const", bufs=1) as constp,
            tc.tile_pool(name="gath", bufs=2) as gathp,
            tc.tile_pool(name="stile", bufs=2) as stilep,
            tc.tile_pool(name="poolps", bufs=2, space="PSUM") as poolpsp,
            tc.tile_pool(name="convps", bufs=4, space="PSUM") as convpsp,
        ):
            # ---- resident constants
            wa1_t = constp.tile([128, 9 * 128], mdt)
            wb1_t = constp.tile([64, 9 * 128], mdt)
            w2p_t = constp.tile([128, 9 * 128], mdt)
            w3p_t = constp.tile([128, 9 * 64], mdt)
            nc.sync.dma_start(out=wa1_t[:, :], in_=wa1[:, :])
            nc.sync.dma_start(out=wb1_t[:, :], in_=wb1[:, :])
            nc.sync.dma_start(out=w2p_t[:, :], in_=w2p[:, :])
            nc.sync.dma_start(out=w3p_t[:, :], in_=w3p[:, :])
            b1_t = constp.tile([128, 1], f32)
            b2_t = constp.tile([128, 1], f32)
            b3_t = constp.tile([64, 1], f32)
            nc.sync.dma_start(out=b1_t[:, :], in_=b1t[:, :])
            nc.sync.dma_start(out=b2_t[:, :], in_=b2t[:, :])
            nc.sync.dma_start(out=b3_t[:, :], in_=b3t[:, :])
            idx_t = constp.tile([128, 8 * Gtot], mybir.dt.int16)
            nc.sync.dma_start(out=idx_t[:, :], in_=idxs_pack[:, :])

            # ---- persistent ring tiles; guards + pad columns zeroed once
            def ring(name, p, w, n=3):
                ts = [constp.tile([p, w], mdt, name=f"{name}{i}")
                      for i in range(n)]
                for t in ts:
                    if half:
                        nc.vector.memset(t[:, :], 0.0)
                    else:
                        nc.vector.memset(t[:, :].bitcast(f32), 0.0)
                return ts

            P_ring = ring("Pr", 128, PGW)
            F_ring = ring("Fr", 64, PGW)
            H1_ring = ring("H1r", 128, PGW)
            H2_ring = ring("H2r", 128, PGW)
            OT_ring = [constp.tile([128, PINT], f32, name=f"OTr{i}")
                       for i in range(3)]

            def conv_layer(x_tiles, w_tiles, Ks, M, bias, out_tile,
                           out_guarded, out_rowbase=0):
                # fully-contiguous ACT evacuation: pad columns receive
                # garbage; guarded out tiles get them re-zeroed below, OT
                # pads are stripped by the host
                r0 = 0
                for rows in CHUNK_ROWS:
                    Nc = rows * PW
                    ps = convpsp.tile([128, 512], f32, tag="convps",
                                      name="cps")
                    nmm = 9 * len(x_tiles)
                    i = 0
                    for ky in range(3):
                        for kx in range(3):
                            t = 3 * ky + kx
                            delta = PW * (ky - 1) + (kx - 1)
                            a = PGUARD + PW * r0 + delta
                            for xt, wt, K in zip(x_tiles, w_tiles, Ks):
                                nc.tensor.matmul(
                                    ps[:M, :Nc],
                                    wt[:K, t * M:(t + 1) * M],
                                    xt[:K, a:a + Nc],
                                    start=(i == 0), stop=(i == nmm - 1),
                                )
                                i += 1
                    off = (PGUARD if out_guarded else 0) + PW * r0
                    nc.scalar.activation(
                        out_tile[out_rowbase:out_rowbase + M,
                                 off:off + Nc],
                        ps[:M, :Nc], LRELU, bias=bias[:M, :], alpha=0.1)
                    r0 += rows
                if out_guarded:
                    # restore the shared zero columns the ACT just dirtied
                    pad_ap = out_tile[:M, PGUARD:PGUARD + PINT].rearrange(
                        "p (r c) -> p r c", c=PW)[:, :, 32:33]
                    nc.vector.memset(
                        pad_ap if half else pad_ap.bitcast(f32), 0.0)

            _state = {}
            for s in range(nslots):
                g = int(G[s])
                do_pool = parts in ("all", "pool")
                do_conv = parts in ("all", "conv")
                P = P_ring[s % 3]
                F = F_ring[s % 3]
                OT = OT_ring[(s // 2) % 3]
                ot_base = 64 * (s % 2)

                if do_pool:
                    if s % 2 == 0:
                        gsum = int(G[s: s + 2].sum())
                        base = int(goff[s])
                        gath = gathp.tile([128, gb_max * 1024], mdt,
                                          tag="gath", name="gath")
                        nc.gpsimd.dma_gather(
                            out_ap=gath[:, :gsum * 1024].rearrange(
                                "p (gg f) -> p gg f", f=1024),
                            in_ap=feats_all[:, :],
                            idxs_ap=idx_t[:, base * 8:(base + gsum) * 8],
                            num_idxs=gsum * 128,
                            num_idxs_reg=gsum * 128,
                            elem_size=1024,
                            single_packet=False,
                        )
                        _state["gath"] = gath
                    if s % 4 == 0:
                        ssum = int(G[s: s + 4].sum())
                        sbase = int(goff[s])
                        s_t = stilep.tile([128, sb_max * 128], mdt,
                                          tag="stile", name="s_mat")
                        nc.sync.dma_start(
                            out=s_t[:, :ssum * 128].rearrange(
                                "p (gg m) -> p gg m", m=128),
                            in_=s_pack[
                                sbase * 128:(sbase + ssum) * 128,
                                :].rearrange("(gg p) m -> p gg m", p=128),
                        )
                        _state["s_t"] = s_t
                    gath = _state["gath"]
                    s_t = _state["s_t"]
                    boff = int(goff[s]) - int(goff[2 * (s // 2)])
                    soff = int(goff[s]) - int(goff[4 * (s // 4)])
                    pool_ps = poolpsp.tile([128, 1024], f32, tag="poolps",
                                           name="pool_ps")
                    for w0 in (0, 512):
                        for gg in range(g):
                            nc.tensor.matmul(
                                pool_ps[:, w0:w0 + 512],
                                s_t[:, (soff + gg) * 128:
                                    (soff + gg + 1) * 128],
                                gath[:, (boff + gg) * 1024 + w0:
                                     (boff + gg) * 1024 + w0 + 512],
                                start=(gg == 0), stop=(gg == g - 1),
                            )
                    # evacuate compact pool PSUM into padded P interior
                    nc.vector.tensor_copy(
                        out=P[:, PGUARD:PGUARD + PINT].rearrange(
                            "p (r c) -> p r c", c=PW)[:, :, 0:32],
                        in_=pool_ps[:, :].rearrange(
                            "p (r c) -> p r c", c=32),
                    )

                nc.sync.dma_start(
                    out=F[:, PGUARD:PGUARD + PINT],
                    in_=feats_own_p[s * 64:(s + 1) * 64, :])

                if do_conv:
                    H1 = H1_ring[s % 3]
                    H2 = H2_ring[s % 3]
                    conv_layer([P, F], [wa1_t, wb1_t], [128, 64], 128,
                               b1_t, H1, True)
                    conv_layer([H1], [w2p_t], [128], 128, b2_t, H2, True)
                    conv_layer([H2], [w3p_t], [128], 64, b3_t, OT, False,
                               out_rowbase=ot_base)
                elif do_pool:
                    nc.vector.tensor_copy(
                        out=OT[ot_base:ot_base + 64, :1024],
                        in_=pool_ps[:64, :])
                else:
                    nc.vector.memset(OT[ot_base:ot_base + 64, :], 0.0)

                if s % 2 == 1 or s == nslots - 1:
                    nrows = ot_base + 64
                    row0 = (s // 2) * 128
                    nc.sync.dma_start(
                        out=out_own[row0:row0 + nrows, :],
                        in_=OT[:nrows, :])
    nc.finalize()
    return nc


# ------------------------------------------------------------- entry point

def kernel(feats, edges, W1, b1, W2, b2, W3, b3):
    import sys
    if "/opt/trn_rl_repo" not in sys.path:
        sys.path.insert(0, "/opt/trn_rl_repo")
    from concourse.bass_utils import run_bass_kernel_spmd

    in_maps, node_lists, G, goff, Gtot = _host_prep(
        feats, edges, W1, b1, W2, b2, W3, b3)
    nc = _build_program_v2(G, goff, Gtot)
    try:
        res = run_bass_kernel_spmd(nc, in_maps, core_ids=list(range(NCORES)))
    except Exception:
        # transient axon tunnel failures (e.g. "mesh desynced") are
        # recoverable on a fresh dispatch
        res = run_bass_kernel_spmd(nc, in_maps, core_ids=list(range(NCORES)))
    global LAST_EXEC_TIME_NS
    LAST_EXEC_TIME_NS = res.exec_time_ns
    out = np.zeros((V, C, H, W), np.float32)
    for c in range(NCORES):
        oo = np.asarray(res.results[c]["out_own"]).reshape(QPC, 64, PINT)
        for s in range(QPC):
            for n_local, v in enumerate(node_lists[c][s]):
                out[v] = oo[s, 16 * n_local:16 * n_local + 16].reshape(
                    16, 32, PW)[:, :, :32]
    return out

